# revision 22
# baseline (speedup 1.0000x reference)
"""Trainium2 Bass kernel for nn_BlockAttnResTransformerBlock (v5).

Same math/structure as v4 (all-bf16, unnormalized softmax, S2 precompute,
PE h-builds + GEMMs, M=32-padded dots) plus SOFTWARE PIPELINING: tile t+1's
front half (DMA, dots, ssq, logits, diag weights) is emitted between tile
t's phase 1 and phase 2, so each engine's in-order queue interleaves the
two tiles and the cross-engine dependency stalls get filled.
"""

import numpy as np
import ml_dtypes
from contextlib import ExitStack

import concourse.bass as bass
import concourse.bacc as bacc
import concourse.tile as tile
from concourse import mybir
from concourse.bass_utils import run_bass_kernel_spmd
from concourse.masks import make_identity

bf16 = ml_dtypes.bfloat16

N_BLK = 8
B, T, D = 4, 2048, 1024
NCORES = 8
TOK = B * T
TPC = TOK // NCORES          # 1024 tokens per core
NT = TPC // 128              # 8 token-tiles per core
EPS = 1e-6
INV_SCALE = 1.0 / 32.0       # 1/sqrt(D)

_BF = mybir.dt.bfloat16
_F32 = mybir.dt.float32

_CACHE = {}


def build_nc():
    nc = bacc.Bacc("TRN2", target_bir_lowering=False, debug=False)

    vn = nc.dram_tensor("vn", [NT, 128, N_BLK, D], _BF, kind="ExternalInput")
    vt = nc.dram_tensor("vt", [NT, 128, 8, N_BLK, 128], _BF,
                        kind="ExternalInput")
    pb = nc.dram_tensor("pb", [NT, 128, D], _BF, kind="ExternalInput")
    qp = nc.dram_tensor("qp", [128, 8, 32], _BF, kind="ExternalInput")
    qa = nc.dram_tensor("qa", [D], _BF, kind="ExternalInput")
    qm = nc.dram_tensor("qm", [D], _BF, kind="ExternalInput")
    wa = nc.dram_tensor("wa", [128, 8, D], _BF, kind="ExternalInput")
    wm = nc.dram_tensor("wm", [128, 8, D], _BF, kind="ExternalInput")
    out = nc.dram_tensor("out", [NT, 128, D], _BF, kind="ExternalOutput")

    AF = mybir.ActivationFunctionType
    AX = mybir.AxisListType
    OP = mybir.AluOpType

    with tile.TileContext(nc) as tc, ExitStack() as ctx:
        consts = ctx.enter_context(tc.tile_pool(name="consts", bufs=1))
        vin = ctx.enter_context(tc.tile_pool(name="vin", bufs=3))
        stats = ctx.enter_context(tc.tile_pool(name="stats", bufs=4))
        work = ctx.enter_context(tc.tile_pool(name="work", bufs=2))
        pdots = ctx.enter_context(tc.tile_pool(name="pdots", bufs=1, space="PSUM"))
        ptp = ctx.enter_context(tc.tile_pool(name="ptp", bufs=1, space="PSUM"))
        ps2 = ctx.enter_context(tc.tile_pool(name="ps2", bufs=1, space="PSUM"))
        ph1 = ctx.enter_context(tc.tile_pool(name="ph1", bufs=1, space="PSUM"))
        pg = ctx.enter_context(tc.tile_pool(name="pg", bufs=1, space="PSUM"))

        ident = consts.tile([128, 128], _BF)
        make_identity(nc, ident)
        eps_sb = consts.tile([128, 1], _F32)
        nc.vector.memset(eps_sb, EPS)
        qp_sb = consts.tile([128, 8, 32], _BF)
        nc.sync.dma_start(out=qp_sb, in_=qp[:, :, :])
        wa_sb = consts.tile([128, 8, D], _BF)
        nc.sync.dma_start(out=wa_sb, in_=wa[:, :, :])
        wm_sb = consts.tile([128, 8, D], _BF)
        nc.sync.dma_start(out=wm_sb, in_=wm[:, :, :])

        def bcast(dst, src_dram):
            ap = src_dram[:]
            nc.sync.dma_start(out=dst, in_=bass.AP(
                tensor=ap.tensor, offset=ap.offset, ap=[[0, 128]] + list(ap.ap)))

        qa_bc = consts.tile([128, D], _BF)
        bcast(qa_bc, qa)
        qm_bc = consts.tile([128, D], _BF)
        bcast(qm_bc, qm)

        def emit_A(tt):
            """Front half of tile tt: DMA, dots, ssq, logits, diag weights."""
            s = {}
            v_sb = vin.tile([128, N_BLK, D], _BF, tag="v")
            s["v"] = v_sb
            nc.sync.dma_start(out=v_sb, in_=vn[tt])
            vt_sb = vin.tile([128, 8, N_BLK, 128], _BF, tag="vt")
            nc.sync.dma_start(out=vt_sb, in_=vt[tt])
            pb_sb = vin.tile([128, D], _BF, tag="pb")
            s["pb"] = pb_sb
            nc.sync.dma_start(out=pb_sb, in_=pb[tt])

            # dots (PE, two rounds through one PSUM bank)
            dsb = stats.tile([32, N_BLK, 128], _BF, tag="dsb")
            for g in range(2):
                dps = pdots.tile([32, 4, 128], _F32, tag="dps")
                for c in range(8):
                    nc.tensor.matmul(dps[0:32, :, :],
                                     lhsT=qp_sb[:, c, :],
                                     rhs=vt_sb[:, c, 4 * g:4 * g + 4, :],
                                     start=(c == 0), stop=(c == 7))
                nc.scalar.activation(out=dsb[:, 4 * g:4 * g + 4, :],
                                     in_=dps, func=AF.Copy)
            dtp = ptp.tile([128, N_BLK, 32], _F32, tag="dtp")
            for n in range(N_BLK):
                nc.tensor.matmul(dtp[:, n, :], lhsT=dsb[0:32, n, :],
                                 rhs=ident[0:32, 0:32], start=True, stop=True)

            # per-block ssq (ACT x6 + DVE x2 + p0 on DVE)
            ssqt = stats.tile([128, 10], _F32, tag="ssqt")
            jA = work.tile([128, D], _BF, tag="jA")
            jC = work.tile([128, D], _BF, tag="jC")
            s["jA"], s["jC"] = jA, jC
            for n in (0, 1, 2, 3, 4):
                nc.scalar.activation(out=jA, in_=v_sb[:, n, :], func=AF.Square,
                                     accum_out=ssqt[:, n:n + 1])
            for n in (5, 6, 7):
                nc.vector.tensor_mul(out=jC, in0=v_sb[:, n, :],
                                     in1=v_sb[:, n, :])
                nc.vector.tensor_reduce(out=ssqt[:, n:n + 1], in_=jC,
                                        axis=AX.X, op=OP.add)
            nc.vector.tensor_mul(out=jC, in0=pb_sb, in1=pb_sb)
            nc.vector.tensor_reduce(out=ssqt[:, 8:9], in_=jC, axis=AX.X,
                                    op=OP.add)
            dot_p0 = stats.tile([128, 1], _F32, tag="dotp0")
            nc.vector.tensor_mul(out=jC, in0=pb_sb, in1=qa_bc)
            nc.vector.tensor_reduce(out=dot_p0, in_=jC, axis=AX.X, op=OP.add)

            sq9 = stats.tile([128, 9], _F32, tag="sq9")
            nc.scalar.activation(out=sq9, in_=ssqt[:, 0:9], func=AF.Sqrt,
                                 scale=1.0 / D, bias=eps_sb[:, :])
            rinv = stats.tile([128, 9], _F32, tag="rinv")
            nc.vector.reciprocal(out=rinv, in_=sq9)

            lg1 = stats.tile([128, 9], _F32, tag="lg1")
            nc.vector.tensor_mul(out=lg1[:, 0:8], in0=dtp[:, :, 0],
                                 in1=rinv[:, 0:8])
            nc.vector.tensor_mul(out=lg1[:, 8:9], in0=dot_p0,
                                 in1=rinv[:, 8:9])
            m1 = stats.tile([128, 1], _F32, tag="m1")
            nc.vector.reduce_max(out=m1, in_=lg1, axis=AX.X)
            mb1 = stats.tile([128, 1], _F32, tag="mb1")
            nc.vector.tensor_scalar_mul(out=mb1, in0=m1, scalar1=-INV_SCALE)
            e1 = stats.tile([128, 9], _F32, tag="e1")
            nc.scalar.activation(out=e1, in_=lg1, func=AF.Exp,
                                 scale=INV_SCALE, bias=mb1[:, :])

            lg2 = stats.tile([128, 8], _F32, tag="lg2")
            nc.vector.tensor_mul(out=lg2, in0=dtp[:, :, 1], in1=rinv[:, 0:8])
            m2c = stats.tile([128, 1], _F32, tag="m2c")
            nc.vector.reduce_max(out=m2c, in_=lg2, axis=AX.X)
            mb2 = stats.tile([128, 1], _F32, tag="mb2")
            s["mb2"] = mb2
            nc.vector.tensor_scalar_mul(out=mb2, in0=m2c, scalar1=-INV_SCALE)
            e2 = stats.tile([128, 8], _F32, tag="e2")
            nc.scalar.activation(out=e2, in_=lg2, func=AF.Exp,
                                 scale=INV_SCALE, bias=mb2[:, :])

            dg1 = work.tile([128, 9, 128], _BF, tag="dg1")
            s["dg1"] = dg1
            for n in range(9):
                nc.vector.tensor_scalar_mul(out=dg1[:, n, :], in0=ident,
                                            scalar1=e1[:, n:n + 1])
            dg2 = work.tile([128, N_BLK, 128], _BF, tag="dg2")
            s["dg2"] = dg2
            for n in range(N_BLK):
                nc.vector.tensor_scalar_mul(out=dg2[:, n, :], in0=ident,
                                            scalar1=e2[:, n:n + 1])
            return s

        def emit_h1(s):
            """h1 = sum e1_n V_n on PE (emitted early to fill PE gaps)."""
            v_sb, pb_sb, dg1 = s["v"], s["pb"], s["dg1"]
            h1p = ph1.tile([128, D], _F32, tag="h1")
            s["h1p"] = h1p
            for n in range(9):
                for half in range(2):
                    sl = slice(512 * half, 512 * half + 512)
                    rhs = v_sb[:, n, sl] if n < 8 else pb_sb[:, sl]
                    nc.tensor.matmul(h1p[:, sl], lhsT=dg1[:, n, :], rhs=rhs,
                                     start=(n == 0), stop=(n == 8))

        def emit_B1(s):
            """Phase 1 of a tile: S2, hn1, GEMM1, p1 + its stats."""
            v_sb, pb_sb, dg2 = s["v"], s["pb"], s["dg2"]
            jA, jC = s["jA"], s["jC"]
            h1p = s["h1p"]
            s2 = ps2.tile([128, D], _F32, tag="s2")
            s["s2"] = s2
            for n in range(N_BLK):
                for half in range(2):
                    sl = slice(512 * half, 512 * half + 512)
                    nc.tensor.matmul(s2[:, sl], lhsT=dg2[:, n, :],
                                     rhs=v_sb[:, n, sl], start=(n == 0),
                                     stop=False, skip_group_check=True)

            ssqh1 = stats.tile([128, 1], _F32, tag="ssqh1")
            nc.scalar.activation(out=jA, in_=h1p, func=AF.Square,
                                 accum_out=ssqh1)
            sqh1 = stats.tile([128, 1], _F32, tag="sqh1")
            nc.scalar.activation(out=sqh1, in_=ssqh1, func=AF.Sqrt,
                                 scale=1.0 / D, bias=eps_sb[:, :])
            rh1 = stats.tile([128, 1], _F32, tag="rh1")
            nc.vector.reciprocal(out=rh1, in_=sqh1)
            hn1 = work.tile([128, D], _BF, tag="hn1")
            nc.scalar.activation(out=hn1, in_=h1p, func=AF.Copy,
                                 scale=rh1[:, :])

            hnT = work.tile([128, 8, 128], _BF, tag="hnT")
            nc.sync.dma_start_transpose(hnT, hn1)
            g1 = pg.tile([128, D], _F32, tag="g")
            for c in range(8):
                for half in range(2):
                    sl = slice(512 * half, 512 * half + 512)
                    nc.tensor.matmul(g1[:, sl], lhsT=hnT[:, c, :],
                                     rhs=wa_sb[:, c, sl],
                                     start=(c == 0), stop=(c == 7))

            p1b = work.tile([128, D], _BF, tag="p1b")
            s["p1b"] = p1b
            nc.vector.tensor_add(out=p1b, in0=g1, in1=pb_sb)
            nc.vector.tensor_mul(out=jC, in0=p1b, in1=p1b)
            ssqp1 = stats.tile([128, 1], _F32, tag="ssqp1")
            nc.vector.tensor_reduce(out=ssqp1, in_=jC, axis=AX.X, op=OP.add)
            dot_p1 = stats.tile([128, 1], _F32, tag="dotp1")
            nc.vector.tensor_mul(out=jC, in0=p1b, in1=qm_bc)
            nc.vector.tensor_reduce(out=dot_p1, in_=jC, axis=AX.X, op=OP.add)
            sqp1 = stats.tile([128, 1], _F32, tag="sqp1")
            nc.scalar.activation(out=sqp1, in_=ssqp1, func=AF.Sqrt,
                                 scale=1.0 / D, bias=eps_sb[:, :])
            rp1 = stats.tile([128, 1], _F32, tag="rp1")
            nc.vector.reciprocal(out=rp1, in_=sqp1)
            l28 = stats.tile([128, 1], _F32, tag="l28")
            nc.vector.tensor_mul(out=l28, in0=dot_p1, in1=rp1)
            w8t = stats.tile([128, 1], _F32, tag="w8t")
            nc.scalar.activation(out=w8t, in_=l28, func=AF.Exp,
                                 scale=INV_SCALE, bias=s["mb2"][:, :])
            dw8 = work.tile([128, 128], _BF, tag="dw8")
            s["dw8"] = dw8
            nc.vector.tensor_scalar_mul(out=dw8, in0=ident, scalar1=w8t[:, :])

        def emit_B2a(s):
            """h2 accumulation finish (PE)."""
            s2, p1b, dw8 = s["s2"], s["p1b"], s["dw8"]
            for half in range(2):
                sl = slice(512 * half, 512 * half + 512)
                nc.tensor.matmul(s2[:, sl], lhsT=dw8, rhs=p1b[:, sl],
                                 start=False, stop=(half == 1),
                                 skip_group_check=True)

        def emit_B2b(s, tt):
            """Phase 2 rest: hn2, GEMM2, output."""
            s2, p1b, jA = s["s2"], s["p1b"], s["jA"]
            ssqh2 = stats.tile([128, 1], _F32, tag="ssqh2")
            nc.scalar.activation(out=jA, in_=s2, func=AF.Square,
                                 accum_out=ssqh2)
            sqh2 = stats.tile([128, 1], _F32, tag="sqh2")
            nc.scalar.activation(out=sqh2, in_=ssqh2, func=AF.Sqrt,
                                 scale=1.0 / D, bias=eps_sb[:, :])
            rh2 = stats.tile([128, 1], _F32, tag="rh2")
            nc.vector.reciprocal(out=rh2, in_=sqh2)
            hn2 = work.tile([128, D], _BF, tag="hn2")
            nc.scalar.activation(out=hn2, in_=s2, func=AF.Copy,
                                 scale=rh2[:, :])

            hnT2 = work.tile([128, 8, 128], _BF, tag="hnT2")
            nc.sync.dma_start_transpose(hnT2, hn2)
            g2 = pg.tile([128, D], _F32, tag="g")
            for c in range(8):
                for half in range(2):
                    sl = slice(512 * half, 512 * half + 512)
                    nc.tensor.matmul(g2[:, sl], lhsT=hnT2[:, c, :],
                                     rhs=wm_sb[:, c, sl],
                                     start=(c == 0), stop=(c == 7))
            out_sb = work.tile([128, D], _BF, tag="outsb")
            nc.vector.tensor_add(out=out_sb, in0=g2, in1=p1b)
            nc.sync.dma_start(out=out[tt], in_=out_sb)

        cur = emit_A(0)
        for tt in range(NT):
            emit_h1(cur)
            emit_B1(cur)
            nxt = emit_A(tt + 1) if tt + 1 < NT else None
            emit_B2a(cur)
            emit_B2b(cur, tt)
            cur = nxt

    nc.compile()
    return nc


def _get_nc():
    if "nc" not in _CACHE:
        _CACHE["nc"] = build_nc()
    return _CACHE["nc"]


def _prepare_in_maps(completed_blocks, partial_block, attn_norm_w, attn_w,
                     mlp_norm_w, mlp_w, attn_res_query, attn_res_norm_w,
                     mlp_res_query, mlp_res_norm_w):
    V = np.ascontiguousarray(np.asarray(completed_blocks, np.float32)).reshape(N_BLK, TOK, D)
    P = np.ascontiguousarray(np.asarray(partial_block, np.float32)).reshape(TOK, D)
    qwa = np.asarray(attn_res_query, np.float32) * np.asarray(attn_res_norm_w, np.float32)
    qwm = np.asarray(mlp_res_query, np.float32) * np.asarray(mlp_res_norm_w, np.float32)
    WaT = (np.asarray(attn_w, np.float32) * np.asarray(attn_norm_w, np.float32)[None, :]).T
    WmT = (np.asarray(mlp_w, np.float32) * np.asarray(mlp_norm_w, np.float32)[None, :]).T

    qp_host = np.zeros((128, 8, 32), bf16)
    qp_host[:, :, 0:2] = (np.stack([qwa, qwm], axis=-1).reshape(8, 128, 2)
                          .transpose(1, 0, 2).astype(bf16))
    wa_host = np.ascontiguousarray(
        WaT.reshape(8, 128, D).transpose(1, 0, 2).astype(bf16))
    wm_host = np.ascontiguousarray(
        WmT.reshape(8, 128, D).transpose(1, 0, 2).astype(bf16))
    qa_host = np.ascontiguousarray(qwa.astype(bf16))
    qm_host = np.ascontiguousarray(qwm.astype(bf16))

    in_maps = []
    for core in range(NCORES):
        sl = slice(core * TPC, (core + 1) * TPC)
        Vc = V[:, sl, :].astype(bf16)                          # [n, 1024, 1024]
        vn_host = np.ascontiguousarray(
            Vc.reshape(N_BLK, NT, 128, D).transpose(1, 2, 0, 3))
        vt_host = np.ascontiguousarray(
            Vc.reshape(N_BLK, NT, 128, 8, 128).transpose(1, 4, 3, 0, 2))
        pb_host = np.ascontiguousarray(P[sl].reshape(NT, 128, D).astype(bf16))
        in_maps.append(dict(vn=vn_host, vt=vt_host, pb=pb_host,
                            qp=qp_host, qa=qa_host, qm=qm_host,
                            wa=wa_host, wm=wm_host))
    return in_maps


def _run(in_maps, **kw):
    nc = _get_nc()
    return run_bass_kernel_spmd(nc, in_maps, core_ids=list(range(NCORES)), **kw)


def kernel(completed_blocks, partial_block, attn_norm_w, attn_w, mlp_norm_w,
           mlp_w, attn_res_query, attn_res_norm_w, mlp_res_query,
           mlp_res_norm_w, layer_in_block=None, **_unused):
    in_maps = _prepare_in_maps(completed_blocks, partial_block, attn_norm_w,
                               attn_w, mlp_norm_w, mlp_w, attn_res_query,
                               attn_res_norm_w, mlp_res_query, mlp_res_norm_w)
    res = _run(in_maps)
    outs = [np.asarray(r["out"], np.float32).reshape(TPC, D) for r in res.results]
    return np.concatenate(outs, axis=0).reshape(B, T, D)
